# revision 18
# baseline (speedup 1.0000x reference)
"""Cost-volume kernel for Trainium2 (8 NeuronCores, data-parallel over B*H rows).

cost[b,h,w,d] = mean_c left[b,h,w,c] * right[b,h,w-(d+1),c], 0 where w-d-1 < 0
Shapes: B=4, H=256, W=512, C=64, D=64 (f32).

Strategy per core (128 independent (b,h) rows):
  - Host casts inputs to real bf16 and pre-transposes to [C, rows*W]
    (left pre-scaled by 1/C), halving input DMA traffic vs f32 and making
    loads plain contiguous DMAs (no transpose-DMA).  "right" is loaded into
    a per-row padded SBUF layout [C, rows*(64+W)] whose 64-col pads are
    zeroed on-chip, so every matmul window is uniform and w-d-1 < 0 yields
    exact zeros straight from the matmul.
  - TensorE: per 128-w block, two M=64/K=64/N=127 matmuls packed into psum
    partition halves via tile_position, rhs windows shifted by 64: psum
    [128, 127] rect per block with the band at col (p mod 64) + d'
    (d' = 63 - d).
  - DVE/ACT evict psum (f32 -> bf16) into a per-group SBUF rect; the rect
    is stored contiguously (full-rate DMA) to per-group DRAM scratch, and a
    DRAM->DRAM DMA with a sheared (flat-affine, +1-elem-per-row) source AP
    pulls out the band G[w, d'] = rect[p, 127*m + (p mod 64) + d'].
    (SBUF-side DMA descriptors cannot express the per-partition shear -
    DRAM-side flat addressing can.)
  - DMA issue is split across the two HWDGE queues (SP=sync, ACT=scalar).
  - Host flips d' -> d and casts bf16 -> f32 during unsharding.
"""

import numpy as np

N_CORES = 8
B_FULL, H_FULL, W, C = 4, 256, 512, 64
D = 64
ROWS = B_FULL * H_FULL           # 1024 independent rows
ROWS_PER_CORE = ROWS // N_CORES  # 128
NB = W // 128                    # 128-w blocks per row
RECT = 127                       # rect cols per block (band at q + d')
SEG = W + 64                     # per-row padded segment in Rt


def build_v3(rows=ROWS_PER_CORE, rg=8, lt_bufs=3, rect_bufs=3, ps_bufs=6,
             load_eng="sync", store_eng="sync", diag_eng="scalar",
             pad_eng="gpsimd", ev_engs=("vector", "scalar"), repeat=1,
             skip=()):
    import concourse.bass as bass
    import concourse.mybir as mybir
    import concourse.tile as tile
    from concourse import bacc

    nc = bacc.Bacc()
    left = nc.declare_dram_parameter("left", [C, rows * W], mybir.dt.bfloat16,
                                     isOutput=False)
    right = nc.declare_dram_parameter("right", [C, rows * W], mybir.dt.bfloat16,
                                      isOutput=False)
    out = nc.declare_dram_parameter("out", [rows * W, D], mybir.dt.bfloat16,
                                    isOutput=True)

    ng = rows // rg
    nblocks = rg * NB              # rect blocks per group
    bcols = nblocks * RECT         # rect buffer bf16 cols
    scr = [nc.dram_tensor(f"scr{g}", [128, bcols], mybir.dt.bfloat16)
           for g in range(ng)]

    with tile.TileContext(nc) as tc:
        with (
            tc.tile_pool(name="lt", bufs=lt_bufs) as lt_pool,
            tc.tile_pool(name="rt", bufs=lt_bufs) as rt_pool,
            tc.tile_pool(name="rect", bufs=rect_bufs) as rect_pool,
            tc.tile_pool(name="ps", bufs=ps_bufs, space="PSUM") as psum_pool,
        ):
          for _rep in range(repeat):
            for g in range(ng):
                c0 = g * rg * W
                Lt = lt_pool.tile([C, rg * W], mybir.dt.bfloat16, tag="lt")
                Rt = rt_pool.tile([C, rg * SEG], mybir.dt.bfloat16, tag="rt")
                if "loads" not in skip:
                    getattr(nc, load_eng).dma_start(Lt[:, :],
                                                    left[:, c0:c0 + rg * W])
                # zero the 64-col pads, then drop each row after its pad
                for r in range(rg):
                    getattr(nc, pad_eng).memset(
                        Rt[:, r * SEG: r * SEG + 64], 0.0)
                rap = Rt[:, :]
                rdst = bass.AP(rap.tensor, rap.offset + 64,
                               [[rg * SEG, C], [SEG, rg], [1, W]])
                rsap = right[:, c0:c0 + rg * W]
                rsrc = bass.AP(rsap.tensor, rsap.offset,
                               [[rows * W, C], [W, rg], [1, W]])
                getattr(nc, load_eng).dma_start(rdst, rsrc)

                Brect = rect_pool.tile([128, bcols], mybir.dt.bfloat16,
                                       tag="rect")
                for r in range(rg):
                    for half in range(NB // 2):
                        # two 128-w blocks share one psum tile (col halves);
                        # sub s covers w = 128i + 64s + q on partitions
                        # [64s, 64s+64); rhs window w' in [w0s - 64, w0s + 63)
                        P = psum_pool.tile([128, 2 * RECT], mybir.dt.float32,
                                           tag="ps")
                        for bi in range(2):
                            i = 2 * half + bi
                            for s in range(2):
                                nc.tensor.matmul(
                                    P[64 * s:64 * (s + 1),
                                      bi * RECT:(bi + 1) * RECT],
                                    Lt[:, r * W + 128 * i + 64 * s:
                                       r * W + 128 * i + 64 * (s + 1)],
                                    Rt[:, r * SEG + 128 * i + 64 * s:
                                       r * SEG + 128 * i + 64 * s + RECT],
                                    start=True, stop=True,
                                    tile_position=(0, 64 * s))
                        m0 = r * NB + 2 * half
                        ev = ev_engs[(r * (NB // 2) + half) % len(ev_engs)]
                        if ev == "vector":
                            nc.vector.tensor_copy(
                                Brect[:, m0 * RECT:(m0 + 2) * RECT], P[:, :])
                        else:
                            nc.scalar.copy(
                                Brect[:, m0 * RECT:(m0 + 2) * RECT], P[:, :])

                # rect -> DRAM scratch (contiguous, full-rate), then band
                # extraction via sheared DRAM->DRAM:
                # out[row0*W + 128m + 64h + q, d'] = scr[64h + q, 127m + q + d']
                sap = scr[g][:, :]
                oap = out[g * rg * W:(g + 1) * rg * W, :]
                if "store" not in skip:
                    getattr(nc, store_eng).dma_start(scr[g][:, :], Brect[:, :])
                for h in range(2 if "extract" not in skip else 0):
                    src = bass.AP(sap.tensor,
                                  sap.offset + 64 * h * bcols,
                                  [[bcols + 1, 64], [RECT, nblocks], [1, D]])
                    dst = bass.AP(oap.tensor,
                                  oap.offset + 64 * h * D,
                                  [[D, 64], [128 * D, nblocks], [1, D]])
                    getattr(nc, diag_eng).dma_start(dst, src)

    nc.compile()
    return nc


def build_v4(rows=ROWS_PER_CORE, rg=64, lg=8, lt_bufs=3, rect_bufs=2,
             ps_bufs=6, load_eng="sync", ex_engs=("sync", "scalar"),
             pad_eng="gpsimd", ev_engs=("vector", "scalar"), repeat=1,
             nbp=2, skip=()):
    """Scratch-free variant: per-q direct band extraction.

    For fixed q = p mod 64 the band sits at constant rect col q + d', so one
    SBUF->DRAM DMA per q (partitions {q, q+64}, all blocks of a big group)
    writes the band straight to `out` - no DRAM scratch roundtrip.  64 DMAs
    per rect group; rect groups are large (rg rows) to amortize issue cost.
    """
    import concourse.bass as bass
    import concourse.mybir as mybir
    import concourse.tile as tile
    from concourse import bacc

    nc = bacc.Bacc()
    left = nc.declare_dram_parameter("left", [C, rows * W], mybir.dt.bfloat16,
                                     isOutput=False)
    right = nc.declare_dram_parameter("right", [C, rows * W], mybir.dt.bfloat16,
                                      isOutput=False)
    out = nc.declare_dram_parameter("out", [rows * W, D], mybir.dt.bfloat16,
                                    isOutput=True)

    ng = rows // rg               # big rect groups
    nsub = rg // lg               # load subgroups per rect group
    nblocks = rg * NB             # rect blocks per group
    bcols = nblocks * RECT

    with tile.TileContext(nc) as tc:
        with (
            tc.tile_pool(name="lt", bufs=lt_bufs) as lt_pool,
            tc.tile_pool(name="rt", bufs=lt_bufs) as rt_pool,
            tc.tile_pool(name="rect", bufs=rect_bufs) as rect_pool,
            tc.tile_pool(name="ps", bufs=ps_bufs, space="PSUM") as psum_pool,
        ):
          for _rep in range(repeat):
            for g in range(ng):
                Brect = rect_pool.tile([128, bcols], mybir.dt.bfloat16,
                                       tag="rect")
                for sub in range(nsub):
                    c0 = (g * rg + sub * lg) * W
                    Lt = lt_pool.tile([C, lg * W], mybir.dt.bfloat16, tag="lt")
                    Rt = rt_pool.tile([C, lg * SEG], mybir.dt.bfloat16,
                                      tag="rt")
                    if "loads" not in skip:
                        getattr(nc, load_eng).dma_start(
                            Lt[:, :], left[:, c0:c0 + lg * W])
                    for r in range(lg):
                        getattr(nc, pad_eng).memset(
                            Rt[:, r * SEG: r * SEG + 64], 0.0)
                    rap = Rt[:, :]
                    rdst = bass.AP(rap.tensor, rap.offset + 64,
                                   [[lg * SEG, C], [SEG, lg], [1, W]])
                    rsap = right[:, c0:c0 + lg * W]
                    rsrc = bass.AP(rsap.tensor, rsap.offset,
                                   [[rows * W, C], [W, lg], [1, W]])
                    if "loads" not in skip:
                        getattr(nc, load_eng).dma_start(rdst, rsrc)

                    for r in range(lg):
                        for half in range(NB // nbp):
                            P = psum_pool.tile([128, nbp * RECT],
                                               mybir.dt.float32, tag="ps")
                            for bi in range(nbp):
                                i = nbp * half + bi
                                for s in range(2):
                                    nc.tensor.matmul(
                                        P[64 * s:64 * (s + 1),
                                          bi * RECT:(bi + 1) * RECT],
                                        Lt[:, r * W + 128 * i + 64 * s:
                                           r * W + 128 * i + 64 * (s + 1)],
                                        Rt[:, r * SEG + 128 * i + 64 * s:
                                           r * SEG + 128 * i + 64 * s + RECT],
                                        start=True, stop=True,
                                        tile_position=(0, 64 * s))
                            m0 = (sub * lg + r) * NB + nbp * half
                            ev = ev_engs[(r * (NB // nbp) + half)
                                         % len(ev_engs)]
                            if ev == "vector":
                                nc.vector.tensor_copy(
                                    Brect[:, m0 * RECT:(m0 + nbp) * RECT],
                                    P[:, :])
                            else:
                                nc.scalar.copy(
                                    Brect[:, m0 * RECT:(m0 + nbp) * RECT],
                                    P[:, :])

                # per-q band write: out[128m + q + 64h, d'] =
                # Brect[q + 64h, 127m + q + d']
                bap = Brect[:, :]
                oap = out[g * rg * W:(g + 1) * rg * W, :]
                for q in range(64 if "extract" not in skip else 0):
                    src = bass.AP(bap.tensor, bap.offset + q * bcols + q,
                                  [[64 * bcols, 2], [RECT, nblocks], [1, D]])
                    dst = bass.AP(oap.tensor, oap.offset + q * D,
                                  [[64 * D, 2], [128 * D, nblocks], [1, D]])
                    getattr(nc, ex_engs[q % len(ex_engs)]).dma_start(dst, src)

    nc.compile()
    return nc


def build_v6(rows=ROWS_PER_CORE, rg=64, lg=8, lt_bufs=3, rect_bufs=2,
             ps_bufs=8, load_eng="sync", ex_engs=("scalar", "gpsimd", "sync"),
             pad_eng="gpsimd", ev_engs=("vector", "scalar"), repeat=1, nbp=4,
             skip=()):
    """v4 + software-pipelined extract issue: the 64 per-q band DMAs of rect
    group g-1 are emitted interleaved between group g's compute chunks (one
    per chunk), so their semaphore waits are satisfied by the time the SEQ
    decodes them and they never head-of-line-block the issuing queue."""
    import concourse.bass as bass
    import concourse.mybir as mybir
    import concourse.tile as tile
    from concourse import bacc

    nc = bacc.Bacc()
    left = nc.declare_dram_parameter("left", [C, rows * W], mybir.dt.bfloat16,
                                     isOutput=False)
    right = nc.declare_dram_parameter("right", [C, rows * W], mybir.dt.bfloat16,
                                      isOutput=False)
    out = nc.declare_dram_parameter("out", [rows * W, D], mybir.dt.bfloat16,
                                    isOutput=True)

    ng = rows // rg
    nsub = rg // lg
    nblocks = rg * NB
    bcols = nblocks * RECT
    chunks_per_group = nsub * (lg * NB // nbp)
    assert chunks_per_group >= 64

    with tile.TileContext(nc) as tc:
        with (
            tc.tile_pool(name="lt", bufs=lt_bufs) as lt_pool,
            tc.tile_pool(name="rt", bufs=lt_bufs) as rt_pool,
            tc.tile_pool(name="rect", bufs=rect_bufs) as rect_pool,
            tc.tile_pool(name="ps", bufs=ps_bufs, space="PSUM") as psum_pool,
        ):
          pending = []            # extract DMAs of the previous rect group

          def emit_extracts(bap, oap):
              ex = []
              for q in range(64 if "extract" not in skip else 0):
                  src = bass.AP(bap.tensor, bap.offset + q * bcols + q,
                                [[64 * bcols, 2], [RECT, nblocks], [1, D]])
                  dst = bass.AP(oap.tensor, oap.offset + q * D,
                                [[64 * D, 2], [128 * D, nblocks], [1, D]])
                  ex.append((ex_engs[q % len(ex_engs)], dst, src))
              return ex

          for _rep in range(repeat):
            for g in range(ng):
                Brect = rect_pool.tile([128, bcols], mybir.dt.bfloat16,
                                       tag="rect")
                ch_no = 0
                for sub in range(nsub):
                    c0 = (g * rg + sub * lg) * W
                    Lt = lt_pool.tile([C, lg * W], mybir.dt.bfloat16, tag="lt")
                    Rt = rt_pool.tile([C, lg * SEG], mybir.dt.bfloat16,
                                      tag="rt")
                    le = ((load_eng,) if isinstance(load_eng, str)
                          else load_eng)
                    if "loads" not in skip:
                        getattr(nc, le[0]).dma_start(
                            Lt[:, :], left[:, c0:c0 + lg * W])
                    for r in range(lg):
                        getattr(nc, pad_eng).memset(
                            Rt[:, r * SEG: r * SEG + 64], 0.0)
                    rap = Rt[:, :]
                    rdst = bass.AP(rap.tensor, rap.offset + 64,
                                   [[lg * SEG, C], [SEG, lg], [1, W]])
                    rsap = right[:, c0:c0 + lg * W]
                    rsrc = bass.AP(rsap.tensor, rsap.offset,
                                   [[rows * W, C], [W, lg], [1, W]])
                    if "loads" not in skip:
                        getattr(nc, le[-1]).dma_start(rdst, rsrc)

                    for ch in range(lg * NB // nbp):
                        P = psum_pool.tile([128, nbp * RECT],
                                           mybir.dt.float32, tag="ps")
                        for bi in range(nbp):
                            rr, i = divmod(ch * nbp + bi, NB)
                            for s in range(2):
                                nc.tensor.matmul(
                                    P[64 * s:64 * (s + 1),
                                      bi * RECT:(bi + 1) * RECT],
                                    Lt[:, rr * W + 128 * i + 64 * s:
                                       rr * W + 128 * i + 64 * (s + 1)],
                                    Rt[:, rr * SEG + 128 * i + 64 * s:
                                       rr * SEG + 128 * i + 64 * s + RECT],
                                    start=True, stop=True,
                                    tile_position=(0, 64 * s))
                        m0 = sub * lg * NB + ch * nbp
                        ev = ev_engs[(sub * (lg * NB // nbp) + ch)
                                     % len(ev_engs)]
                        if ev == "vector":
                            nc.vector.tensor_copy(
                                Brect[:, m0 * RECT:(m0 + nbp) * RECT], P[:, :])
                        else:
                            nc.scalar.copy(
                                Brect[:, m0 * RECT:(m0 + nbp) * RECT], P[:, :])
                        if pending:
                            eng, dst, src = pending.pop()
                            getattr(nc, eng).dma_start(dst, src)
                        ch_no += 1

                for eng, dst, src in pending:
                    getattr(nc, eng).dma_start(dst, src)
                pending = emit_extracts(
                    Brect[:, :], out[g * rg * W:(g + 1) * rg * W, :])

          for eng, dst, src in pending:
              getattr(nc, eng).dma_start(dst, src)

    nc.compile()
    return nc


def build_v5(rows=ROWS_PER_CORE, rg=64, lg=16, lt_bufs=2, rect_bufs=2,
             ps_bufs=4, load_eng="sync", ex_engs=("scalar", "gpsimd"),
             ev_engs=("vector", "scalar"), repeat=1, nbp=8, skip=()):
    """v4 + host-padded right ([C, rows*SEG], zeros in the 64-col pads) so
    loads are plain 2D DMAs with no on-chip memsets, and nbp blocks per psum
    tile on a 128-col grid (in-bank quadrants) with strided-src evicts."""
    import concourse.bass as bass
    import concourse.mybir as mybir
    import concourse.tile as tile
    from concourse import bacc

    nc = bacc.Bacc()
    left = nc.declare_dram_parameter("left", [C, rows * W], mybir.dt.bfloat16,
                                     isOutput=False)
    right = nc.declare_dram_parameter("right", [C, rows * SEG],
                                      mybir.dt.bfloat16, isOutput=False)
    out = nc.declare_dram_parameter("out", [rows * W, D], mybir.dt.bfloat16,
                                    isOutput=True)

    ng = rows // rg               # big rect groups
    nsub = rg // lg               # load subgroups per rect group
    nblocks = rg * NB             # rect blocks per group
    bcols = nblocks * RECT
    PB = 128                      # psum col grid per block

    with tile.TileContext(nc) as tc:
        with (
            tc.tile_pool(name="lt", bufs=lt_bufs) as lt_pool,
            tc.tile_pool(name="rt", bufs=lt_bufs) as rt_pool,
            tc.tile_pool(name="rect", bufs=rect_bufs) as rect_pool,
            tc.tile_pool(name="ps", bufs=ps_bufs, space="PSUM") as psum_pool,
        ):
          for _rep in range(repeat):
            for g in range(ng):
                Brect = rect_pool.tile([128, bcols], mybir.dt.bfloat16,
                                       tag="rect")
                for sub in range(nsub):
                    r0 = g * rg + sub * lg
                    Lt = lt_pool.tile([C, lg * W], mybir.dt.bfloat16, tag="lt")
                    Rt = rt_pool.tile([C, lg * SEG], mybir.dt.bfloat16,
                                      tag="rt")
                    if "loads" not in skip:
                        getattr(nc, load_eng).dma_start(
                            Lt[:, :], left[:, r0 * W:(r0 + lg) * W])
                        getattr(nc, load_eng).dma_start(
                            Rt[:, :], right[:, r0 * SEG:(r0 + lg) * SEG])

                    nchunks = (lg * NB) // nbp
                    for ch in range(nchunks):
                        P = psum_pool.tile([128, nbp * PB],
                                           mybir.dt.float32, tag="ps")
                        for bi in range(nbp):
                            rr, i = divmod(ch * nbp + bi, NB)
                            for s in range(2):
                                nc.tensor.matmul(
                                    P[64 * s:64 * (s + 1),
                                      bi * PB:bi * PB + RECT],
                                    Lt[:, rr * W + 128 * i + 64 * s:
                                       rr * W + 128 * i + 64 * (s + 1)],
                                    Rt[:, rr * SEG + 128 * i + 64 * s:
                                       rr * SEG + 128 * i + 64 * s + RECT],
                                    start=True, stop=True,
                                    tile_position=(0, 64 * s))
                        m0 = sub * lg * NB + ch * nbp
                        bap0 = Brect[:, m0 * RECT:(m0 + nbp) * RECT]
                        pap = P[:, :]
                        src = bass.AP(pap.tensor, pap.offset,
                                      [[nbp * PB, 128], [PB, nbp], [1, RECT]])
                        dst = bass.AP(bap0.tensor, bap0.offset,
                                      [[bcols, 128], [RECT, nbp], [1, RECT]])
                        ev = ev_engs[ch % len(ev_engs)]
                        if ev == "vector":
                            nc.vector.tensor_copy(dst, src)
                        else:
                            nc.scalar.copy(dst, src)

                # per-q band write: out[128m + q + 64h, d'] =
                # Brect[q + 64h, 127m + q + d']
                bap = Brect[:, :]
                oap = out[g * rg * W:(g + 1) * rg * W, :]
                for q in range(64 if "extract" not in skip else 0):
                    src = bass.AP(bap.tensor, bap.offset + q * bcols + q,
                                  [[64 * bcols, 2], [RECT, nblocks], [1, D]])
                    dst = bass.AP(oap.tensor, oap.offset + q * D,
                                  [[64 * D, 2], [128 * D, nblocks], [1, D]])
                    getattr(nc, ex_engs[q % len(ex_engs)]).dma_start(dst, src)

    nc.compile()
    return nc


def _to_bf16_t(x, scale, pad=0):
    """[rows, W, C] f32 -> [C, rows*(pad+W)] bf16, zeros in the pad cols."""
    import ml_dtypes
    if scale != 1.0:
        x = x * scale
    xt = np.ascontiguousarray(x.transpose(2, 0, 1)).astype(ml_dtypes.bfloat16)
    if pad:
        rows = xt.shape[1]
        padded = np.zeros((C, rows, pad + W), dtype=ml_dtypes.bfloat16)
        padded[:, :, pad:] = xt
        xt = padded
    return xt.reshape(C, -1)


def make_in_maps(seed=0, pad_right=False):
    """Random prepped per-core in_maps (bench harness helper)."""
    rng = np.random.default_rng(seed)
    lf = rng.standard_normal((ROWS, W, C), dtype=np.float32)
    rf = rng.standard_normal((ROWS, W, C), dtype=np.float32)
    in_maps = []
    for k in range(N_CORES):
        sl = slice(k * ROWS_PER_CORE, (k + 1) * ROWS_PER_CORE)
        in_maps.append({
            "left": _to_bf16_t(lf[sl], 1.0 / C),
            "right": _to_bf16_t(rf[sl], 1.0, pad=64 if pad_right else 0),
        })
    return in_maps


def in_map_to_rows(m):
    """Recover [ROWS_PER_CORE, W, C] f32 (prepped) arrays from an in_map."""
    lf = np.asarray(m["left"]).astype(np.float32)
    rf = np.asarray(m["right"]).astype(np.float32)
    lf = lf.reshape(C, ROWS_PER_CORE, W).transpose(1, 2, 0)
    seg = rf.size // (C * ROWS_PER_CORE)
    rf = rf.reshape(C, ROWS_PER_CORE, seg)[:, :, seg - W:].transpose(1, 2, 0)
    return lf, rf


_NC_CACHE = {}


def kernel(left_feature, right_feature):
    from concourse.bass_utils import run_bass_kernel_spmd

    lf = np.asarray(left_feature, dtype=np.float32).reshape(ROWS, W, C)
    rf = np.asarray(right_feature, dtype=np.float32).reshape(ROWS, W, C)

    if "nc" not in _NC_CACHE:
        _NC_CACHE["nc"] = build_v6(load_eng=("sync", "scalar"))
    nc = _NC_CACHE["nc"]

    in_maps = []
    for k in range(N_CORES):
        sl = slice(k * ROWS_PER_CORE, (k + 1) * ROWS_PER_CORE)
        in_maps.append({
            "left": _to_bf16_t(lf[sl], 1.0 / C),
            "right": _to_bf16_t(rf[sl], 1.0),
        })

    res = run_bass_kernel_spmd(nc, in_maps, core_ids=list(range(N_CORES)))

    out = np.empty((ROWS, W, D), dtype=np.float32)
    for k in range(N_CORES):
        g = res.results[k]["out"].astype(np.float32).reshape(
            ROWS_PER_CORE, W, D)
        out[k * ROWS_PER_CORE:(k + 1) * ROWS_PER_CORE] = g[:, :, ::-1]
    return out.reshape(B_FULL, H_FULL, W, D)


# revision 20
# speedup vs baseline: 2.1448x; 2.1448x over previous
"""Cost-volume kernel for Trainium2 (8 NeuronCores, data-parallel over B*H rows).

cost[b,h,w,d] = mean_c left[b,h,w,c] * right[b,h,w-(d+1),c], 0 where w-d-1 < 0
Shapes: B=4, H=256, W=512, C=64, D=64 (f32).

Strategy per core (128 independent (b,h) rows):
  - Host casts inputs to real bf16 and pre-transposes to [C, rows*W]
    (left pre-scaled by 1/C), halving input DMA traffic vs f32 and making
    loads plain contiguous DMAs (no transpose-DMA).  "right" is loaded into
    a per-row padded SBUF layout [C, rows*(64+W)] whose 64-col pads are
    zeroed on-chip, so every matmul window is uniform and w-d-1 < 0 yields
    exact zeros straight from the matmul.
  - TensorE: per 128-w block, two M=64/K=64/N=127 matmuls packed into psum
    partition halves via tile_position, rhs windows shifted by 64: psum
    [128, 127] rect per block with the band at col (p mod 64) + d'
    (d' = 63 - d).
  - DVE/ACT evict psum (f32 -> bf16) into a per-group SBUF rect; the rect
    is stored contiguously (full-rate DMA) to per-group DRAM scratch, and a
    DRAM->DRAM DMA with a sheared (flat-affine, +1-elem-per-row) source AP
    pulls out the band G[w, d'] = rect[p, 127*m + (p mod 64) + d'].
    (SBUF-side DMA descriptors cannot express the per-partition shear -
    DRAM-side flat addressing can.)
  - DMA issue is split across the two HWDGE queues (SP=sync, ACT=scalar).
  - Host flips d' -> d and casts bf16 -> f32 during unsharding.
"""

import numpy as np

N_CORES = 8
B_FULL, H_FULL, W, C = 4, 256, 512, 64
D = 64
ROWS = B_FULL * H_FULL           # 1024 independent rows
ROWS_PER_CORE = ROWS // N_CORES  # 128
NB = W // 128                    # 128-w blocks per row
RECT = 127                       # rect cols per block (band at q + d')
SEG = W + 64                     # per-row padded segment in Rt


def build_v3(rows=ROWS_PER_CORE, rg=8, lt_bufs=3, rect_bufs=3, ps_bufs=6,
             load_eng="sync", store_eng="sync", diag_eng="scalar",
             pad_eng="gpsimd", ev_engs=("vector", "scalar"), repeat=1,
             skip=()):
    import concourse.bass as bass
    import concourse.mybir as mybir
    import concourse.tile as tile
    from concourse import bacc

    nc = bacc.Bacc()
    left = nc.declare_dram_parameter("left", [C, rows * W], mybir.dt.bfloat16,
                                     isOutput=False)
    right = nc.declare_dram_parameter("right", [C, rows * W], mybir.dt.bfloat16,
                                      isOutput=False)
    out = nc.declare_dram_parameter("out", [rows * W, D], mybir.dt.bfloat16,
                                    isOutput=True)

    ng = rows // rg
    nblocks = rg * NB              # rect blocks per group
    bcols = nblocks * RECT         # rect buffer bf16 cols
    scr = [nc.dram_tensor(f"scr{g}", [128, bcols], mybir.dt.bfloat16)
           for g in range(ng)]

    with tile.TileContext(nc) as tc:
        with (
            tc.tile_pool(name="lt", bufs=lt_bufs) as lt_pool,
            tc.tile_pool(name="rt", bufs=lt_bufs) as rt_pool,
            tc.tile_pool(name="rect", bufs=rect_bufs) as rect_pool,
            tc.tile_pool(name="ps", bufs=ps_bufs, space="PSUM") as psum_pool,
        ):
          for _rep in range(repeat):
            for g in range(ng):
                c0 = g * rg * W
                Lt = lt_pool.tile([C, rg * W], mybir.dt.bfloat16, tag="lt")
                Rt = rt_pool.tile([C, rg * SEG], mybir.dt.bfloat16, tag="rt")
                if "loads" not in skip:
                    getattr(nc, load_eng).dma_start(Lt[:, :],
                                                    left[:, c0:c0 + rg * W])
                # zero the 64-col pads, then drop each row after its pad
                for r in range(rg):
                    getattr(nc, pad_eng).memset(
                        Rt[:, r * SEG: r * SEG + 64], 0.0)
                rap = Rt[:, :]
                rdst = bass.AP(rap.tensor, rap.offset + 64,
                               [[rg * SEG, C], [SEG, rg], [1, W]])
                rsap = right[:, c0:c0 + rg * W]
                rsrc = bass.AP(rsap.tensor, rsap.offset,
                               [[rows * W, C], [W, rg], [1, W]])
                getattr(nc, load_eng).dma_start(rdst, rsrc)

                Brect = rect_pool.tile([128, bcols], mybir.dt.bfloat16,
                                       tag="rect")
                for r in range(rg):
                    for half in range(NB // 2):
                        # two 128-w blocks share one psum tile (col halves);
                        # sub s covers w = 128i + 64s + q on partitions
                        # [64s, 64s+64); rhs window w' in [w0s - 64, w0s + 63)
                        P = psum_pool.tile([128, 2 * RECT], mybir.dt.float32,
                                           tag="ps")
                        for bi in range(2):
                            i = 2 * half + bi
                            for s in range(2):
                                nc.tensor.matmul(
                                    P[64 * s:64 * (s + 1),
                                      bi * RECT:(bi + 1) * RECT],
                                    Lt[:, r * W + 128 * i + 64 * s:
                                       r * W + 128 * i + 64 * (s + 1)],
                                    Rt[:, r * SEG + 128 * i + 64 * s:
                                       r * SEG + 128 * i + 64 * s + RECT],
                                    start=True, stop=True,
                                    tile_position=(0, 64 * s))
                        m0 = r * NB + 2 * half
                        ev = ev_engs[(r * (NB // 2) + half) % len(ev_engs)]
                        if ev == "vector":
                            nc.vector.tensor_copy(
                                Brect[:, m0 * RECT:(m0 + 2) * RECT], P[:, :])
                        else:
                            nc.scalar.copy(
                                Brect[:, m0 * RECT:(m0 + 2) * RECT], P[:, :])

                # rect -> DRAM scratch (contiguous, full-rate), then band
                # extraction via sheared DRAM->DRAM:
                # out[row0*W + 128m + 64h + q, d'] = scr[64h + q, 127m + q + d']
                sap = scr[g][:, :]
                oap = out[g * rg * W:(g + 1) * rg * W, :]
                if "store" not in skip:
                    getattr(nc, store_eng).dma_start(scr[g][:, :], Brect[:, :])
                for h in range(2 if "extract" not in skip else 0):
                    src = bass.AP(sap.tensor,
                                  sap.offset + 64 * h * bcols,
                                  [[bcols + 1, 64], [RECT, nblocks], [1, D]])
                    dst = bass.AP(oap.tensor,
                                  oap.offset + 64 * h * D,
                                  [[D, 64], [128 * D, nblocks], [1, D]])
                    getattr(nc, diag_eng).dma_start(dst, src)

    nc.compile()
    return nc


def build_v4(rows=ROWS_PER_CORE, rg=64, lg=8, lt_bufs=3, rect_bufs=2,
             ps_bufs=6, load_eng="sync", ex_engs=("sync", "scalar"),
             pad_eng="gpsimd", ev_engs=("vector", "scalar"), repeat=1,
             nbp=2, skip=()):
    """Scratch-free variant: per-q direct band extraction.

    For fixed q = p mod 64 the band sits at constant rect col q + d', so one
    SBUF->DRAM DMA per q (partitions {q, q+64}, all blocks of a big group)
    writes the band straight to `out` - no DRAM scratch roundtrip.  64 DMAs
    per rect group; rect groups are large (rg rows) to amortize issue cost.
    """
    import concourse.bass as bass
    import concourse.mybir as mybir
    import concourse.tile as tile
    from concourse import bacc

    nc = bacc.Bacc()
    left = nc.declare_dram_parameter("left", [C, rows * W], mybir.dt.bfloat16,
                                     isOutput=False)
    right = nc.declare_dram_parameter("right", [C, rows * W], mybir.dt.bfloat16,
                                      isOutput=False)
    out = nc.declare_dram_parameter("out", [rows * W, D], mybir.dt.bfloat16,
                                    isOutput=True)

    ng = rows // rg               # big rect groups
    nsub = rg // lg               # load subgroups per rect group
    nblocks = rg * NB             # rect blocks per group
    bcols = nblocks * RECT

    with tile.TileContext(nc) as tc:
        with (
            tc.tile_pool(name="lt", bufs=lt_bufs) as lt_pool,
            tc.tile_pool(name="rt", bufs=lt_bufs) as rt_pool,
            tc.tile_pool(name="rect", bufs=rect_bufs) as rect_pool,
            tc.tile_pool(name="ps", bufs=ps_bufs, space="PSUM") as psum_pool,
        ):
          for _rep in range(repeat):
            for g in range(ng):
                Brect = rect_pool.tile([128, bcols], mybir.dt.bfloat16,
                                       tag="rect")
                for sub in range(nsub):
                    c0 = (g * rg + sub * lg) * W
                    Lt = lt_pool.tile([C, lg * W], mybir.dt.bfloat16, tag="lt")
                    Rt = rt_pool.tile([C, lg * SEG], mybir.dt.bfloat16,
                                      tag="rt")
                    if "loads" not in skip:
                        getattr(nc, load_eng).dma_start(
                            Lt[:, :], left[:, c0:c0 + lg * W])
                    for r in range(lg):
                        getattr(nc, pad_eng).memset(
                            Rt[:, r * SEG: r * SEG + 64], 0.0)
                    rap = Rt[:, :]
                    rdst = bass.AP(rap.tensor, rap.offset + 64,
                                   [[lg * SEG, C], [SEG, lg], [1, W]])
                    rsap = right[:, c0:c0 + lg * W]
                    rsrc = bass.AP(rsap.tensor, rsap.offset,
                                   [[rows * W, C], [W, lg], [1, W]])
                    if "loads" not in skip:
                        getattr(nc, load_eng).dma_start(rdst, rsrc)

                    for r in range(lg):
                        for half in range(NB // nbp):
                            P = psum_pool.tile([128, nbp * RECT],
                                               mybir.dt.float32, tag="ps")
                            for bi in range(nbp):
                                i = nbp * half + bi
                                for s in range(2):
                                    nc.tensor.matmul(
                                        P[64 * s:64 * (s + 1),
                                          bi * RECT:(bi + 1) * RECT],
                                        Lt[:, r * W + 128 * i + 64 * s:
                                           r * W + 128 * i + 64 * (s + 1)],
                                        Rt[:, r * SEG + 128 * i + 64 * s:
                                           r * SEG + 128 * i + 64 * s + RECT],
                                        start=True, stop=True,
                                        tile_position=(0, 64 * s))
                            m0 = (sub * lg + r) * NB + nbp * half
                            ev = ev_engs[(r * (NB // nbp) + half)
                                         % len(ev_engs)]
                            if ev == "vector":
                                nc.vector.tensor_copy(
                                    Brect[:, m0 * RECT:(m0 + nbp) * RECT],
                                    P[:, :])
                            else:
                                nc.scalar.copy(
                                    Brect[:, m0 * RECT:(m0 + nbp) * RECT],
                                    P[:, :])

                # per-q band write: out[128m + q + 64h, d'] =
                # Brect[q + 64h, 127m + q + d']
                bap = Brect[:, :]
                oap = out[g * rg * W:(g + 1) * rg * W, :]
                for q in range(64 if "extract" not in skip else 0):
                    src = bass.AP(bap.tensor, bap.offset + q * bcols + q,
                                  [[64 * bcols, 2], [RECT, nblocks], [1, D]])
                    dst = bass.AP(oap.tensor, oap.offset + q * D,
                                  [[64 * D, 2], [128 * D, nblocks], [1, D]])
                    getattr(nc, ex_engs[q % len(ex_engs)]).dma_start(dst, src)

    nc.compile()
    return nc


def build_v7(rows=ROWS_PER_CORE, rg=8, lt_bufs=3, rect_bufs=3, ps_bufs=8,
             load_eng="sync", store_eng="sync", diag_eng="scalar",
             pad_eng="gpsimd", ev_engs=("vector", "scalar"), repeat=1,
             nbp=4, interleave=True, exorder="q", skip=()):
    """v3 scratch-roundtrip architecture + lessons from v4-v6:
    - nbp blocks per psum tile ([128, nbp*127] f32, single bank for nbp=4)
      with one contiguous evict each (fewer DVE/ACT instructions).
    - store/extract of group g-1 issued interleaved into group g's compute
      chunks so they never head-of-line-block the loads on their queue.
    - store and extract split into partition halves (h=0,1) so extract h
      chains right behind store h.
    - exorder="m" reorders the extract APs m-outer for write combining.
    """
    import concourse.bass as bass
    import concourse.mybir as mybir
    import concourse.tile as tile
    from concourse import bacc

    nc = bacc.Bacc()
    left = nc.declare_dram_parameter("left", [C, rows * W], mybir.dt.bfloat16,
                                     isOutput=False)
    right = nc.declare_dram_parameter("right", [C, rows * W], mybir.dt.bfloat16,
                                      isOutput=False)
    out = nc.declare_dram_parameter("out", [rows * W, D], mybir.dt.bfloat16,
                                    isOutput=True)

    ng = rows // rg
    nblocks = rg * NB
    bcols = nblocks * RECT
    nch = rg * NB // nbp           # compute chunks per group
    scr = [nc.dram_tensor(f"scr{g}", [128, bcols], mybir.dt.bfloat16)
           for g in range(ng)]

    with tile.TileContext(nc) as tc:
        with (
            tc.tile_pool(name="lt", bufs=lt_bufs) as lt_pool,
            tc.tile_pool(name="rt", bufs=lt_bufs) as rt_pool,
            tc.tile_pool(name="rect", bufs=rect_bufs) as rect_pool,
            tc.tile_pool(name="ps", bufs=ps_bufs, space="PSUM") as psum_pool,
        ):
          pending = []

          def emit_sx(Brect, g):
              """Store+extract DMA list for group g (issued later)."""
              sx = []
              bap = Brect[:, :]
              sap = scr[g][:, :]
              oap = out[g * rg * W:(g + 1) * rg * W, :]
              st = (store_eng,) if isinstance(store_eng, str) else store_eng
              dg = (diag_eng,) if isinstance(diag_eng, str) else diag_eng
              for h in range(2):
                  if "store" not in skip:
                      bsrc = bass.AP(bap.tensor, bap.offset + 64 * h * bcols,
                                     [[bcols, 64], [1, bcols]])
                      bdst = bass.AP(sap.tensor, sap.offset + 64 * h * bcols,
                                     [[bcols, 64], [1, bcols]])
                      sx.append((st[h % len(st)], bdst, bsrc))
                  if "extract" not in skip:
                      if exorder == "q":
                          src = bass.AP(sap.tensor, sap.offset + 64 * h * bcols,
                                        [[bcols + 1, 64], [RECT, nblocks],
                                         [1, D]])
                          dst = bass.AP(oap.tensor, oap.offset + 64 * h * D,
                                        [[D, 64], [128 * D, nblocks], [1, D]])
                      else:
                          src = bass.AP(sap.tensor, sap.offset + 64 * h * bcols,
                                        [[RECT, nblocks], [bcols + 1, 64],
                                         [1, D]])
                          dst = bass.AP(oap.tensor, oap.offset + 64 * h * D,
                                        [[128 * D, nblocks], [D, 64], [1, D]])
                      sx.append((dg[h % len(dg)], dst, src))
              return sx

          for _rep in range(repeat):
            for g in range(ng):
                c0 = g * rg * W
                Lt = lt_pool.tile([C, rg * W], mybir.dt.bfloat16, tag="lt")
                Rt = rt_pool.tile([C, rg * SEG], mybir.dt.bfloat16, tag="rt")
                if "loads" not in skip:
                    getattr(nc, load_eng).dma_start(Lt[:, :],
                                                    left[:, c0:c0 + rg * W])
                for r in range(rg):
                    getattr(nc, pad_eng).memset(
                        Rt[:, r * SEG: r * SEG + 64], 0.0)
                rap = Rt[:, :]
                rdst = bass.AP(rap.tensor, rap.offset + 64,
                               [[rg * SEG, C], [SEG, rg], [1, W]])
                rsap = right[:, c0:c0 + rg * W]
                rsrc = bass.AP(rsap.tensor, rsap.offset,
                               [[rows * W, C], [W, rg], [1, W]])
                if "loads" not in skip:
                    getattr(nc, load_eng).dma_start(rdst, rsrc)

                Brect = rect_pool.tile([128, bcols], mybir.dt.bfloat16,
                                       tag="rect")
                npend = len(pending)
                for ch in range(nch):
                    P = psum_pool.tile([128, nbp * RECT], mybir.dt.float32,
                                       tag="ps")
                    for bi in range(nbp):
                        rr, i = divmod(ch * nbp + bi, NB)
                        for s in range(2):
                            nc.tensor.matmul(
                                P[64 * s:64 * (s + 1),
                                  bi * RECT:(bi + 1) * RECT],
                                Lt[:, rr * W + 128 * i + 64 * s:
                                   rr * W + 128 * i + 64 * (s + 1)],
                                Rt[:, rr * SEG + 128 * i + 64 * s:
                                   rr * SEG + 128 * i + 64 * s + RECT],
                                start=True, stop=True,
                                tile_position=(0, 64 * s))
                    m0 = ch * nbp
                    ev = ev_engs[ch % len(ev_engs)]
                    if ev == "vector":
                        nc.vector.tensor_copy(
                            Brect[:, m0 * RECT:(m0 + nbp) * RECT], P[:, :])
                    else:
                        nc.scalar.copy(
                            Brect[:, m0 * RECT:(m0 + nbp) * RECT], P[:, :])
                    # drain pending store/extracts of the previous group,
                    # spread over this group's chunks
                    while pending and len(pending) > npend * (nch - 1 - ch) / nch:
                        eng, dst, src = pending.pop(0)
                        getattr(nc, eng).dma_start(dst, src)

                sx = emit_sx(Brect, g)
                if interleave:
                    pending = sx
                else:
                    for eng, dst, src in sx:
                        getattr(nc, eng).dma_start(dst, src)

          for eng, dst, src in pending:
              getattr(nc, eng).dma_start(dst, src)

    nc.compile()
    return nc


def build_v6(rows=ROWS_PER_CORE, rg=64, lg=8, lt_bufs=3, rect_bufs=2,
             ps_bufs=8, load_eng="sync", ex_engs=("scalar", "gpsimd", "sync"),
             pad_eng="gpsimd", ev_engs=("vector", "scalar"), repeat=1, nbp=4,
             skip=()):
    """v4 + software-pipelined extract issue: the 64 per-q band DMAs of rect
    group g-1 are emitted interleaved between group g's compute chunks (one
    per chunk), so their semaphore waits are satisfied by the time the SEQ
    decodes them and they never head-of-line-block the issuing queue."""
    import concourse.bass as bass
    import concourse.mybir as mybir
    import concourse.tile as tile
    from concourse import bacc

    nc = bacc.Bacc()
    left = nc.declare_dram_parameter("left", [C, rows * W], mybir.dt.bfloat16,
                                     isOutput=False)
    right = nc.declare_dram_parameter("right", [C, rows * W], mybir.dt.bfloat16,
                                      isOutput=False)
    out = nc.declare_dram_parameter("out", [rows * W, D], mybir.dt.bfloat16,
                                    isOutput=True)

    ng = rows // rg
    nsub = rg // lg
    nblocks = rg * NB
    bcols = nblocks * RECT
    chunks_per_group = nsub * (lg * NB // nbp)
    assert chunks_per_group >= 64

    with tile.TileContext(nc) as tc:
        with (
            tc.tile_pool(name="lt", bufs=lt_bufs) as lt_pool,
            tc.tile_pool(name="rt", bufs=lt_bufs) as rt_pool,
            tc.tile_pool(name="rect", bufs=rect_bufs) as rect_pool,
            tc.tile_pool(name="ps", bufs=ps_bufs, space="PSUM") as psum_pool,
        ):
          pending = []            # extract DMAs of the previous rect group

          def emit_extracts(bap, oap):
              ex = []
              for q in range(64 if "extract" not in skip else 0):
                  src = bass.AP(bap.tensor, bap.offset + q * bcols + q,
                                [[64 * bcols, 2], [RECT, nblocks], [1, D]])
                  dst = bass.AP(oap.tensor, oap.offset + q * D,
                                [[64 * D, 2], [128 * D, nblocks], [1, D]])
                  ex.append((ex_engs[q % len(ex_engs)], dst, src))
              return ex

          for _rep in range(repeat):
            for g in range(ng):
                Brect = rect_pool.tile([128, bcols], mybir.dt.bfloat16,
                                       tag="rect")
                ch_no = 0
                for sub in range(nsub):
                    c0 = (g * rg + sub * lg) * W
                    Lt = lt_pool.tile([C, lg * W], mybir.dt.bfloat16, tag="lt")
                    Rt = rt_pool.tile([C, lg * SEG], mybir.dt.bfloat16,
                                      tag="rt")
                    le = ((load_eng,) if isinstance(load_eng, str)
                          else load_eng)
                    if "loads" not in skip:
                        getattr(nc, le[0]).dma_start(
                            Lt[:, :], left[:, c0:c0 + lg * W])
                    for r in range(lg):
                        getattr(nc, pad_eng).memset(
                            Rt[:, r * SEG: r * SEG + 64], 0.0)
                    rap = Rt[:, :]
                    rdst = bass.AP(rap.tensor, rap.offset + 64,
                                   [[lg * SEG, C], [SEG, lg], [1, W]])
                    rsap = right[:, c0:c0 + lg * W]
                    rsrc = bass.AP(rsap.tensor, rsap.offset,
                                   [[rows * W, C], [W, lg], [1, W]])
                    if "loads" not in skip:
                        getattr(nc, le[-1]).dma_start(rdst, rsrc)

                    for ch in range(lg * NB // nbp):
                        P = psum_pool.tile([128, nbp * RECT],
                                           mybir.dt.float32, tag="ps")
                        for bi in range(nbp):
                            rr, i = divmod(ch * nbp + bi, NB)
                            for s in range(2):
                                nc.tensor.matmul(
                                    P[64 * s:64 * (s + 1),
                                      bi * RECT:(bi + 1) * RECT],
                                    Lt[:, rr * W + 128 * i + 64 * s:
                                       rr * W + 128 * i + 64 * (s + 1)],
                                    Rt[:, rr * SEG + 128 * i + 64 * s:
                                       rr * SEG + 128 * i + 64 * s + RECT],
                                    start=True, stop=True,
                                    tile_position=(0, 64 * s))
                        m0 = sub * lg * NB + ch * nbp
                        ev = ev_engs[(sub * (lg * NB // nbp) + ch)
                                     % len(ev_engs)]
                        if ev == "vector":
                            nc.vector.tensor_copy(
                                Brect[:, m0 * RECT:(m0 + nbp) * RECT], P[:, :])
                        else:
                            nc.scalar.copy(
                                Brect[:, m0 * RECT:(m0 + nbp) * RECT], P[:, :])
                        if pending:
                            eng, dst, src = pending.pop()
                            getattr(nc, eng).dma_start(dst, src)
                        ch_no += 1

                for eng, dst, src in pending:
                    getattr(nc, eng).dma_start(dst, src)
                pending = emit_extracts(
                    Brect[:, :], out[g * rg * W:(g + 1) * rg * W, :])

          for eng, dst, src in pending:
              getattr(nc, eng).dma_start(dst, src)

    nc.compile()
    return nc


def build_v5(rows=ROWS_PER_CORE, rg=64, lg=16, lt_bufs=2, rect_bufs=2,
             ps_bufs=4, load_eng="sync", ex_engs=("scalar", "gpsimd"),
             ev_engs=("vector", "scalar"), repeat=1, nbp=8, skip=()):
    """v4 + host-padded right ([C, rows*SEG], zeros in the 64-col pads) so
    loads are plain 2D DMAs with no on-chip memsets, and nbp blocks per psum
    tile on a 128-col grid (in-bank quadrants) with strided-src evicts."""
    import concourse.bass as bass
    import concourse.mybir as mybir
    import concourse.tile as tile
    from concourse import bacc

    nc = bacc.Bacc()
    left = nc.declare_dram_parameter("left", [C, rows * W], mybir.dt.bfloat16,
                                     isOutput=False)
    right = nc.declare_dram_parameter("right", [C, rows * SEG],
                                      mybir.dt.bfloat16, isOutput=False)
    out = nc.declare_dram_parameter("out", [rows * W, D], mybir.dt.bfloat16,
                                    isOutput=True)

    ng = rows // rg               # big rect groups
    nsub = rg // lg               # load subgroups per rect group
    nblocks = rg * NB             # rect blocks per group
    bcols = nblocks * RECT
    PB = 128                      # psum col grid per block

    with tile.TileContext(nc) as tc:
        with (
            tc.tile_pool(name="lt", bufs=lt_bufs) as lt_pool,
            tc.tile_pool(name="rt", bufs=lt_bufs) as rt_pool,
            tc.tile_pool(name="rect", bufs=rect_bufs) as rect_pool,
            tc.tile_pool(name="ps", bufs=ps_bufs, space="PSUM") as psum_pool,
        ):
          for _rep in range(repeat):
            for g in range(ng):
                Brect = rect_pool.tile([128, bcols], mybir.dt.bfloat16,
                                       tag="rect")
                for sub in range(nsub):
                    r0 = g * rg + sub * lg
                    Lt = lt_pool.tile([C, lg * W], mybir.dt.bfloat16, tag="lt")
                    Rt = rt_pool.tile([C, lg * SEG], mybir.dt.bfloat16,
                                      tag="rt")
                    if "loads" not in skip:
                        getattr(nc, load_eng).dma_start(
                            Lt[:, :], left[:, r0 * W:(r0 + lg) * W])
                        getattr(nc, load_eng).dma_start(
                            Rt[:, :], right[:, r0 * SEG:(r0 + lg) * SEG])

                    nchunks = (lg * NB) // nbp
                    for ch in range(nchunks):
                        P = psum_pool.tile([128, nbp * PB],
                                           mybir.dt.float32, tag="ps")
                        for bi in range(nbp):
                            rr, i = divmod(ch * nbp + bi, NB)
                            for s in range(2):
                                nc.tensor.matmul(
                                    P[64 * s:64 * (s + 1),
                                      bi * PB:bi * PB + RECT],
                                    Lt[:, rr * W + 128 * i + 64 * s:
                                       rr * W + 128 * i + 64 * (s + 1)],
                                    Rt[:, rr * SEG + 128 * i + 64 * s:
                                       rr * SEG + 128 * i + 64 * s + RECT],
                                    start=True, stop=True,
                                    tile_position=(0, 64 * s))
                        m0 = sub * lg * NB + ch * nbp
                        bap0 = Brect[:, m0 * RECT:(m0 + nbp) * RECT]
                        pap = P[:, :]
                        src = bass.AP(pap.tensor, pap.offset,
                                      [[nbp * PB, 128], [PB, nbp], [1, RECT]])
                        dst = bass.AP(bap0.tensor, bap0.offset,
                                      [[bcols, 128], [RECT, nbp], [1, RECT]])
                        ev = ev_engs[ch % len(ev_engs)]
                        if ev == "vector":
                            nc.vector.tensor_copy(dst, src)
                        else:
                            nc.scalar.copy(dst, src)

                # per-q band write: out[128m + q + 64h, d'] =
                # Brect[q + 64h, 127m + q + d']
                bap = Brect[:, :]
                oap = out[g * rg * W:(g + 1) * rg * W, :]
                for q in range(64 if "extract" not in skip else 0):
                    src = bass.AP(bap.tensor, bap.offset + q * bcols + q,
                                  [[64 * bcols, 2], [RECT, nblocks], [1, D]])
                    dst = bass.AP(oap.tensor, oap.offset + q * D,
                                  [[64 * D, 2], [128 * D, nblocks], [1, D]])
                    getattr(nc, ex_engs[q % len(ex_engs)]).dma_start(dst, src)

    nc.compile()
    return nc


def _to_bf16_t(x, scale, pad=0):
    """[rows, W, C] f32 -> [C, rows*(pad+W)] bf16, zeros in the pad cols."""
    import ml_dtypes
    if scale != 1.0:
        x = x * scale
    xt = np.ascontiguousarray(x.transpose(2, 0, 1)).astype(ml_dtypes.bfloat16)
    if pad:
        rows = xt.shape[1]
        padded = np.zeros((C, rows, pad + W), dtype=ml_dtypes.bfloat16)
        padded[:, :, pad:] = xt
        xt = padded
    return xt.reshape(C, -1)


def make_in_maps(seed=0, pad_right=False):
    """Random prepped per-core in_maps (bench harness helper)."""
    rng = np.random.default_rng(seed)
    lf = rng.standard_normal((ROWS, W, C), dtype=np.float32)
    rf = rng.standard_normal((ROWS, W, C), dtype=np.float32)
    in_maps = []
    for k in range(N_CORES):
        sl = slice(k * ROWS_PER_CORE, (k + 1) * ROWS_PER_CORE)
        in_maps.append({
            "left": _to_bf16_t(lf[sl], 1.0 / C),
            "right": _to_bf16_t(rf[sl], 1.0, pad=64 if pad_right else 0),
        })
    return in_maps


def in_map_to_rows(m):
    """Recover [ROWS_PER_CORE, W, C] f32 (prepped) arrays from an in_map."""
    lf = np.asarray(m["left"]).astype(np.float32)
    rf = np.asarray(m["right"]).astype(np.float32)
    lf = lf.reshape(C, ROWS_PER_CORE, W).transpose(1, 2, 0)
    seg = rf.size // (C * ROWS_PER_CORE)
    rf = rf.reshape(C, ROWS_PER_CORE, seg)[:, :, seg - W:].transpose(1, 2, 0)
    return lf, rf


_NC_CACHE = {}


def kernel(left_feature, right_feature):
    from concourse.bass_utils import run_bass_kernel_spmd

    lf = np.asarray(left_feature, dtype=np.float32).reshape(ROWS, W, C)
    rf = np.asarray(right_feature, dtype=np.float32).reshape(ROWS, W, C)

    if "nc" not in _NC_CACHE:
        _NC_CACHE["nc"] = build_v6(load_eng=("sync", "scalar"))
    nc = _NC_CACHE["nc"]

    in_maps = []
    for k in range(N_CORES):
        sl = slice(k * ROWS_PER_CORE, (k + 1) * ROWS_PER_CORE)
        in_maps.append({
            "left": _to_bf16_t(lf[sl], 1.0 / C),
            "right": _to_bf16_t(rf[sl], 1.0),
        })

    res = run_bass_kernel_spmd(nc, in_maps, core_ids=list(range(N_CORES)))

    out = np.empty((ROWS, W, D), dtype=np.float32)
    for k in range(N_CORES):
        g = res.results[k]["out"].astype(np.float32).reshape(
            ROWS_PER_CORE, W, D)
        out[k * ROWS_PER_CORE:(k + 1) * ROWS_PER_CORE] = g[:, :, ::-1]
    return out.reshape(B_FULL, H_FULL, W, D)


# revision 21
# speedup vs baseline: 2.4365x; 1.1360x over previous
"""Cost-volume kernel for Trainium2 (8 NeuronCores, data-parallel over B*H rows).

cost[b,h,w,d] = mean_c left[b,h,w,c] * right[b,h,w-(d+1),c], 0 where w-d-1 < 0
Shapes: B=4, H=256, W=512, C=64, D=64 (f32).

Strategy per core (128 independent (b,h) rows):
  - Host casts inputs to real bf16 and pre-transposes to [C, rows*W]
    (left pre-scaled by 1/C), halving input DMA traffic vs f32 and making
    loads plain contiguous DMAs (no transpose-DMA).  "right" is loaded into
    a per-row padded SBUF layout [C, rows*(64+W)] whose 64-col pads are
    zeroed on-chip, so every matmul window is uniform and w-d-1 < 0 yields
    exact zeros straight from the matmul.
  - TensorE: per 128-w block, two M=64/K=64/N=127 matmuls packed into psum
    partition halves via tile_position, rhs windows shifted by 64: psum
    [128, 127] rect per block with the band at col (p mod 64) + d'
    (d' = 63 - d).
  - DVE/ACT evict psum (f32 -> bf16) into a per-group SBUF rect; the rect
    is stored contiguously (full-rate DMA) to per-group DRAM scratch, and a
    DRAM->DRAM DMA with a sheared (flat-affine, +1-elem-per-row) source AP
    pulls out the band G[w, d'] = rect[p, 127*m + (p mod 64) + d'].
    (SBUF-side DMA descriptors cannot express the per-partition shear -
    DRAM-side flat addressing can.)
  - DMA issue is split across the two HWDGE queues (SP=sync, ACT=scalar).
  - Host flips d' -> d and casts bf16 -> f32 during unsharding.
"""

import numpy as np

N_CORES = 8
B_FULL, H_FULL, W, C = 4, 256, 512, 64
D = 64
ROWS = B_FULL * H_FULL           # 1024 independent rows
ROWS_PER_CORE = ROWS // N_CORES  # 128
NB = W // 128                    # 128-w blocks per row
RECT = 127                       # rect cols per block (band at q + d')
SEG = W + 64                     # per-row padded segment in Rt


def build_v3(rows=ROWS_PER_CORE, rg=8, lt_bufs=3, rect_bufs=3, ps_bufs=6,
             load_eng="sync", store_eng="sync", diag_eng="scalar",
             pad_eng="gpsimd", ev_engs=("vector", "scalar"), repeat=1,
             skip=()):
    import concourse.bass as bass
    import concourse.mybir as mybir
    import concourse.tile as tile
    from concourse import bacc

    nc = bacc.Bacc()
    left = nc.declare_dram_parameter("left", [C, rows * W], mybir.dt.bfloat16,
                                     isOutput=False)
    right = nc.declare_dram_parameter("right", [C, rows * W], mybir.dt.bfloat16,
                                      isOutput=False)
    out = nc.declare_dram_parameter("out", [rows * W, D], mybir.dt.bfloat16,
                                    isOutput=True)

    ng = rows // rg
    nblocks = rg * NB              # rect blocks per group
    bcols = nblocks * RECT         # rect buffer bf16 cols
    scr = [nc.dram_tensor(f"scr{g}", [128, bcols], mybir.dt.bfloat16)
           for g in range(ng)]

    with tile.TileContext(nc) as tc:
        with (
            tc.tile_pool(name="lt", bufs=lt_bufs) as lt_pool,
            tc.tile_pool(name="rt", bufs=lt_bufs) as rt_pool,
            tc.tile_pool(name="rect", bufs=rect_bufs) as rect_pool,
            tc.tile_pool(name="ps", bufs=ps_bufs, space="PSUM") as psum_pool,
        ):
          for _rep in range(repeat):
            for g in range(ng):
                c0 = g * rg * W
                Lt = lt_pool.tile([C, rg * W], mybir.dt.bfloat16, tag="lt")
                Rt = rt_pool.tile([C, rg * SEG], mybir.dt.bfloat16, tag="rt")
                if "loads" not in skip:
                    getattr(nc, load_eng).dma_start(Lt[:, :],
                                                    left[:, c0:c0 + rg * W])
                # zero the 64-col pads, then drop each row after its pad
                for r in range(rg):
                    getattr(nc, pad_eng).memset(
                        Rt[:, r * SEG: r * SEG + 64], 0.0)
                rap = Rt[:, :]
                rdst = bass.AP(rap.tensor, rap.offset + 64,
                               [[rg * SEG, C], [SEG, rg], [1, W]])
                rsap = right[:, c0:c0 + rg * W]
                rsrc = bass.AP(rsap.tensor, rsap.offset,
                               [[rows * W, C], [W, rg], [1, W]])
                getattr(nc, load_eng).dma_start(rdst, rsrc)

                Brect = rect_pool.tile([128, bcols], mybir.dt.bfloat16,
                                       tag="rect")
                for r in range(rg):
                    for half in range(NB // 2):
                        # two 128-w blocks share one psum tile (col halves);
                        # sub s covers w = 128i + 64s + q on partitions
                        # [64s, 64s+64); rhs window w' in [w0s - 64, w0s + 63)
                        P = psum_pool.tile([128, 2 * RECT], mybir.dt.float32,
                                           tag="ps")
                        for bi in range(2):
                            i = 2 * half + bi
                            for s in range(2):
                                nc.tensor.matmul(
                                    P[64 * s:64 * (s + 1),
                                      bi * RECT:(bi + 1) * RECT],
                                    Lt[:, r * W + 128 * i + 64 * s:
                                       r * W + 128 * i + 64 * (s + 1)],
                                    Rt[:, r * SEG + 128 * i + 64 * s:
                                       r * SEG + 128 * i + 64 * s + RECT],
                                    start=True, stop=True,
                                    tile_position=(0, 64 * s))
                        m0 = r * NB + 2 * half
                        ev = ev_engs[(r * (NB // 2) + half) % len(ev_engs)]
                        if ev == "vector":
                            nc.vector.tensor_copy(
                                Brect[:, m0 * RECT:(m0 + 2) * RECT], P[:, :])
                        else:
                            nc.scalar.copy(
                                Brect[:, m0 * RECT:(m0 + 2) * RECT], P[:, :])

                # rect -> DRAM scratch (contiguous, full-rate), then band
                # extraction via sheared DRAM->DRAM:
                # out[row0*W + 128m + 64h + q, d'] = scr[64h + q, 127m + q + d']
                sap = scr[g][:, :]
                oap = out[g * rg * W:(g + 1) * rg * W, :]
                if "store" not in skip:
                    getattr(nc, store_eng).dma_start(scr[g][:, :], Brect[:, :])
                for h in range(2 if "extract" not in skip else 0):
                    src = bass.AP(sap.tensor,
                                  sap.offset + 64 * h * bcols,
                                  [[bcols + 1, 64], [RECT, nblocks], [1, D]])
                    dst = bass.AP(oap.tensor,
                                  oap.offset + 64 * h * D,
                                  [[D, 64], [128 * D, nblocks], [1, D]])
                    getattr(nc, diag_eng).dma_start(dst, src)

    nc.compile()
    return nc


def build_v4(rows=ROWS_PER_CORE, rg=64, lg=8, lt_bufs=3, rect_bufs=2,
             ps_bufs=6, load_eng="sync", ex_engs=("sync", "scalar"),
             pad_eng="gpsimd", ev_engs=("vector", "scalar"), repeat=1,
             nbp=2, skip=()):
    """Scratch-free variant: per-q direct band extraction.

    For fixed q = p mod 64 the band sits at constant rect col q + d', so one
    SBUF->DRAM DMA per q (partitions {q, q+64}, all blocks of a big group)
    writes the band straight to `out` - no DRAM scratch roundtrip.  64 DMAs
    per rect group; rect groups are large (rg rows) to amortize issue cost.
    """
    import concourse.bass as bass
    import concourse.mybir as mybir
    import concourse.tile as tile
    from concourse import bacc

    nc = bacc.Bacc()
    left = nc.declare_dram_parameter("left", [C, rows * W], mybir.dt.bfloat16,
                                     isOutput=False)
    right = nc.declare_dram_parameter("right", [C, rows * W], mybir.dt.bfloat16,
                                      isOutput=False)
    out = nc.declare_dram_parameter("out", [rows * W, D], mybir.dt.bfloat16,
                                    isOutput=True)

    ng = rows // rg               # big rect groups
    nsub = rg // lg               # load subgroups per rect group
    nblocks = rg * NB             # rect blocks per group
    bcols = nblocks * RECT

    with tile.TileContext(nc) as tc:
        with (
            tc.tile_pool(name="lt", bufs=lt_bufs) as lt_pool,
            tc.tile_pool(name="rt", bufs=lt_bufs) as rt_pool,
            tc.tile_pool(name="rect", bufs=rect_bufs) as rect_pool,
            tc.tile_pool(name="ps", bufs=ps_bufs, space="PSUM") as psum_pool,
        ):
          for _rep in range(repeat):
            for g in range(ng):
                Brect = rect_pool.tile([128, bcols], mybir.dt.bfloat16,
                                       tag="rect")
                for sub in range(nsub):
                    c0 = (g * rg + sub * lg) * W
                    Lt = lt_pool.tile([C, lg * W], mybir.dt.bfloat16, tag="lt")
                    Rt = rt_pool.tile([C, lg * SEG], mybir.dt.bfloat16,
                                      tag="rt")
                    if "loads" not in skip:
                        getattr(nc, load_eng).dma_start(
                            Lt[:, :], left[:, c0:c0 + lg * W])
                    for r in range(lg):
                        getattr(nc, pad_eng).memset(
                            Rt[:, r * SEG: r * SEG + 64], 0.0)
                    rap = Rt[:, :]
                    rdst = bass.AP(rap.tensor, rap.offset + 64,
                                   [[lg * SEG, C], [SEG, lg], [1, W]])
                    rsap = right[:, c0:c0 + lg * W]
                    rsrc = bass.AP(rsap.tensor, rsap.offset,
                                   [[rows * W, C], [W, lg], [1, W]])
                    if "loads" not in skip:
                        getattr(nc, load_eng).dma_start(rdst, rsrc)

                    for r in range(lg):
                        for half in range(NB // nbp):
                            P = psum_pool.tile([128, nbp * RECT],
                                               mybir.dt.float32, tag="ps")
                            for bi in range(nbp):
                                i = nbp * half + bi
                                for s in range(2):
                                    nc.tensor.matmul(
                                        P[64 * s:64 * (s + 1),
                                          bi * RECT:(bi + 1) * RECT],
                                        Lt[:, r * W + 128 * i + 64 * s:
                                           r * W + 128 * i + 64 * (s + 1)],
                                        Rt[:, r * SEG + 128 * i + 64 * s:
                                           r * SEG + 128 * i + 64 * s + RECT],
                                        start=True, stop=True,
                                        tile_position=(0, 64 * s))
                            m0 = (sub * lg + r) * NB + nbp * half
                            ev = ev_engs[(r * (NB // nbp) + half)
                                         % len(ev_engs)]
                            if ev == "vector":
                                nc.vector.tensor_copy(
                                    Brect[:, m0 * RECT:(m0 + nbp) * RECT],
                                    P[:, :])
                            else:
                                nc.scalar.copy(
                                    Brect[:, m0 * RECT:(m0 + nbp) * RECT],
                                    P[:, :])

                # per-q band write: out[128m + q + 64h, d'] =
                # Brect[q + 64h, 127m + q + d']
                bap = Brect[:, :]
                oap = out[g * rg * W:(g + 1) * rg * W, :]
                for q in range(64 if "extract" not in skip else 0):
                    src = bass.AP(bap.tensor, bap.offset + q * bcols + q,
                                  [[64 * bcols, 2], [RECT, nblocks], [1, D]])
                    dst = bass.AP(oap.tensor, oap.offset + q * D,
                                  [[64 * D, 2], [128 * D, nblocks], [1, D]])
                    getattr(nc, ex_engs[q % len(ex_engs)]).dma_start(dst, src)

    nc.compile()
    return nc


def build_v7(rows=ROWS_PER_CORE, rg=8, lt_bufs=3, rect_bufs=3, ps_bufs=8,
             load_eng="sync", store_eng="sync", diag_eng="scalar",
             pad_eng="gpsimd", ev_engs=("vector", "scalar"), repeat=1,
             nbp=4, interleave=True, exorder="q", qsplit=False, skip=()):
    """v3 scratch-roundtrip architecture + lessons from v4-v6:
    - nbp blocks per psum tile ([128, nbp*127] f32, single bank for nbp=4)
      with one contiguous evict each (fewer DVE/ACT instructions).
    - store/extract of group g-1 issued interleaved into group g's compute
      chunks so they never head-of-line-block the loads on their queue.
    - store and extract split into partition halves (h=0,1) so extract h
      chains right behind store h.
    - exorder="m" reorders the extract APs m-outer for write combining.
    """
    import concourse.bass as bass
    import concourse.mybir as mybir
    import concourse.tile as tile
    from concourse import bacc

    nc = bacc.Bacc()
    left = nc.declare_dram_parameter("left", [C, rows * W], mybir.dt.bfloat16,
                                     isOutput=False)
    right = nc.declare_dram_parameter("right", [C, rows * W], mybir.dt.bfloat16,
                                      isOutput=False)
    out = nc.declare_dram_parameter("out", [rows * W, D], mybir.dt.bfloat16,
                                    isOutput=True)

    ng = rows // rg
    nblocks = rg * NB
    bcols = nblocks * RECT
    SW = 95                        # stored cols per block under qsplit
    scols = nblocks * SW
    nch = rg * NB // nbp           # compute chunks per group
    scr = [nc.dram_tensor(f"scr{g}", [128, scols if qsplit else bcols],
                          mybir.dt.bfloat16)
           for g in range(ng)]

    with tile.TileContext(nc) as tc:
        with (
            tc.tile_pool(name="lt", bufs=lt_bufs) as lt_pool,
            tc.tile_pool(name="rt", bufs=lt_bufs) as rt_pool,
            tc.tile_pool(name="rect", bufs=rect_bufs) as rect_pool,
            tc.tile_pool(name="ps", bufs=ps_bufs, space="PSUM") as psum_pool,
        ):
          pending = []

          def emit_sx(Brect, g):
              """Store+extract DMA list for group g (issued later)."""
              sx = []
              bap = Brect[:, :]
              sap = scr[g][:, :]
              oap = out[g * rg * W:(g + 1) * rg * W, :]
              st = (store_eng,) if isinstance(store_eng, str) else store_eng
              dg = (diag_eng,) if isinstance(diag_eng, str) else diag_eng
              if qsplit:
                  # store only the 95-col window each partition quarter's
                  # band needs: quarter t = partitions [32t, 32t+32),
                  # window cols 32*(t%2) + [0, 95) of each block.
                  for t in range(4):
                      ct = 32 * (t % 2)
                      if "store" not in skip:
                          bs = bass.AP(bap.tensor,
                                       bap.offset + 32 * t * bcols + ct,
                                       [[bcols, 32], [RECT, nblocks], [1, SW]])
                          sd = bass.AP(sap.tensor, sap.offset + 32 * t * scols,
                                       [[scols, 32], [SW, nblocks], [1, SW]])
                          sx.append((st[t % len(st)], sd, bs))
                      if "extract" not in skip:
                          if exorder == "q":
                              es = bass.AP(sap.tensor,
                                           sap.offset + 32 * t * scols,
                                           [[scols + 1, 32], [SW, nblocks],
                                            [1, D]])
                              ed = bass.AP(oap.tensor, oap.offset + 32 * t * D,
                                           [[D, 32], [128 * D, nblocks],
                                            [1, D]])
                          else:
                              es = bass.AP(sap.tensor,
                                           sap.offset + 32 * t * scols,
                                           [[SW, nblocks], [scols + 1, 32],
                                            [1, D]])
                              ed = bass.AP(oap.tensor, oap.offset + 32 * t * D,
                                           [[128 * D, nblocks], [D, 32],
                                            [1, D]])
                          sx.append((dg[t % len(dg)], ed, es))
                  return sx
              for h in range(2):
                  if "store" not in skip:
                      bsrc = bass.AP(bap.tensor, bap.offset + 64 * h * bcols,
                                     [[bcols, 64], [1, bcols]])
                      bdst = bass.AP(sap.tensor, sap.offset + 64 * h * bcols,
                                     [[bcols, 64], [1, bcols]])
                      sx.append((st[h % len(st)], bdst, bsrc))
                  if "extract" not in skip:
                      if exorder == "q":
                          src = bass.AP(sap.tensor, sap.offset + 64 * h * bcols,
                                        [[bcols + 1, 64], [RECT, nblocks],
                                         [1, D]])
                          dst = bass.AP(oap.tensor, oap.offset + 64 * h * D,
                                        [[D, 64], [128 * D, nblocks], [1, D]])
                      else:
                          src = bass.AP(sap.tensor, sap.offset + 64 * h * bcols,
                                        [[RECT, nblocks], [bcols + 1, 64],
                                         [1, D]])
                          dst = bass.AP(oap.tensor, oap.offset + 64 * h * D,
                                        [[128 * D, nblocks], [D, 64], [1, D]])
                      sx.append((dg[h % len(dg)], dst, src))
              return sx

          for _rep in range(repeat):
            for g in range(ng):
                c0 = g * rg * W
                Lt = lt_pool.tile([C, rg * W], mybir.dt.bfloat16, tag="lt")
                Rt = rt_pool.tile([C, rg * SEG], mybir.dt.bfloat16, tag="rt")
                if "loads" not in skip:
                    getattr(nc, load_eng).dma_start(Lt[:, :],
                                                    left[:, c0:c0 + rg * W])
                for r in range(rg):
                    getattr(nc, pad_eng).memset(
                        Rt[:, r * SEG: r * SEG + 64], 0.0)
                rap = Rt[:, :]
                rdst = bass.AP(rap.tensor, rap.offset + 64,
                               [[rg * SEG, C], [SEG, rg], [1, W]])
                rsap = right[:, c0:c0 + rg * W]
                rsrc = bass.AP(rsap.tensor, rsap.offset,
                               [[rows * W, C], [W, rg], [1, W]])
                if "loads" not in skip:
                    getattr(nc, load_eng).dma_start(rdst, rsrc)

                Brect = rect_pool.tile([128, bcols], mybir.dt.bfloat16,
                                       tag="rect")
                npend = len(pending)
                for ch in range(nch):
                    P = psum_pool.tile([128, nbp * RECT], mybir.dt.float32,
                                       tag="ps")
                    for bi in range(nbp):
                        rr, i = divmod(ch * nbp + bi, NB)
                        for s in range(2):
                            nc.tensor.matmul(
                                P[64 * s:64 * (s + 1),
                                  bi * RECT:(bi + 1) * RECT],
                                Lt[:, rr * W + 128 * i + 64 * s:
                                   rr * W + 128 * i + 64 * (s + 1)],
                                Rt[:, rr * SEG + 128 * i + 64 * s:
                                   rr * SEG + 128 * i + 64 * s + RECT],
                                start=True, stop=True,
                                tile_position=(0, 64 * s))
                    m0 = ch * nbp
                    ev = ev_engs[ch % len(ev_engs)]
                    if ev == "vector":
                        nc.vector.tensor_copy(
                            Brect[:, m0 * RECT:(m0 + nbp) * RECT], P[:, :])
                    else:
                        nc.scalar.copy(
                            Brect[:, m0 * RECT:(m0 + nbp) * RECT], P[:, :])
                    # drain pending store/extracts of the previous group,
                    # spread over this group's chunks
                    while pending and len(pending) > npend * (nch - 1 - ch) / nch:
                        eng, dst, src = pending.pop(0)
                        getattr(nc, eng).dma_start(dst, src)

                sx = emit_sx(Brect, g)
                if interleave:
                    pending = sx
                else:
                    for eng, dst, src in sx:
                        getattr(nc, eng).dma_start(dst, src)

          for eng, dst, src in pending:
              getattr(nc, eng).dma_start(dst, src)

    nc.compile()
    return nc


def build_v6(rows=ROWS_PER_CORE, rg=64, lg=8, lt_bufs=3, rect_bufs=2,
             ps_bufs=8, load_eng="sync", ex_engs=("scalar", "gpsimd", "sync"),
             pad_eng="gpsimd", ev_engs=("vector", "scalar"), repeat=1, nbp=4,
             skip=()):
    """v4 + software-pipelined extract issue: the 64 per-q band DMAs of rect
    group g-1 are emitted interleaved between group g's compute chunks (one
    per chunk), so their semaphore waits are satisfied by the time the SEQ
    decodes them and they never head-of-line-block the issuing queue."""
    import concourse.bass as bass
    import concourse.mybir as mybir
    import concourse.tile as tile
    from concourse import bacc

    nc = bacc.Bacc()
    left = nc.declare_dram_parameter("left", [C, rows * W], mybir.dt.bfloat16,
                                     isOutput=False)
    right = nc.declare_dram_parameter("right", [C, rows * W], mybir.dt.bfloat16,
                                      isOutput=False)
    out = nc.declare_dram_parameter("out", [rows * W, D], mybir.dt.bfloat16,
                                    isOutput=True)

    ng = rows // rg
    nsub = rg // lg
    nblocks = rg * NB
    bcols = nblocks * RECT
    chunks_per_group = nsub * (lg * NB // nbp)
    assert chunks_per_group >= 64

    with tile.TileContext(nc) as tc:
        with (
            tc.tile_pool(name="lt", bufs=lt_bufs) as lt_pool,
            tc.tile_pool(name="rt", bufs=lt_bufs) as rt_pool,
            tc.tile_pool(name="rect", bufs=rect_bufs) as rect_pool,
            tc.tile_pool(name="ps", bufs=ps_bufs, space="PSUM") as psum_pool,
        ):
          pending = []            # extract DMAs of the previous rect group

          def emit_extracts(bap, oap):
              ex = []
              for q in range(64 if "extract" not in skip else 0):
                  src = bass.AP(bap.tensor, bap.offset + q * bcols + q,
                                [[64 * bcols, 2], [RECT, nblocks], [1, D]])
                  dst = bass.AP(oap.tensor, oap.offset + q * D,
                                [[64 * D, 2], [128 * D, nblocks], [1, D]])
                  ex.append((ex_engs[q % len(ex_engs)], dst, src))
              return ex

          for _rep in range(repeat):
            for g in range(ng):
                Brect = rect_pool.tile([128, bcols], mybir.dt.bfloat16,
                                       tag="rect")
                ch_no = 0
                for sub in range(nsub):
                    c0 = (g * rg + sub * lg) * W
                    Lt = lt_pool.tile([C, lg * W], mybir.dt.bfloat16, tag="lt")
                    Rt = rt_pool.tile([C, lg * SEG], mybir.dt.bfloat16,
                                      tag="rt")
                    le = ((load_eng,) if isinstance(load_eng, str)
                          else load_eng)
                    if "loads" not in skip:
                        getattr(nc, le[0]).dma_start(
                            Lt[:, :], left[:, c0:c0 + lg * W])
                    for r in range(lg):
                        getattr(nc, pad_eng).memset(
                            Rt[:, r * SEG: r * SEG + 64], 0.0)
                    rap = Rt[:, :]
                    rdst = bass.AP(rap.tensor, rap.offset + 64,
                                   [[lg * SEG, C], [SEG, lg], [1, W]])
                    rsap = right[:, c0:c0 + lg * W]
                    rsrc = bass.AP(rsap.tensor, rsap.offset,
                                   [[rows * W, C], [W, lg], [1, W]])
                    if "loads" not in skip:
                        getattr(nc, le[-1]).dma_start(rdst, rsrc)

                    for ch in range(lg * NB // nbp):
                        P = psum_pool.tile([128, nbp * RECT],
                                           mybir.dt.float32, tag="ps")
                        for bi in range(nbp):
                            rr, i = divmod(ch * nbp + bi, NB)
                            for s in range(2):
                                nc.tensor.matmul(
                                    P[64 * s:64 * (s + 1),
                                      bi * RECT:(bi + 1) * RECT],
                                    Lt[:, rr * W + 128 * i + 64 * s:
                                       rr * W + 128 * i + 64 * (s + 1)],
                                    Rt[:, rr * SEG + 128 * i + 64 * s:
                                       rr * SEG + 128 * i + 64 * s + RECT],
                                    start=True, stop=True,
                                    tile_position=(0, 64 * s))
                        m0 = sub * lg * NB + ch * nbp
                        ev = ev_engs[(sub * (lg * NB // nbp) + ch)
                                     % len(ev_engs)]
                        if ev == "vector":
                            nc.vector.tensor_copy(
                                Brect[:, m0 * RECT:(m0 + nbp) * RECT], P[:, :])
                        else:
                            nc.scalar.copy(
                                Brect[:, m0 * RECT:(m0 + nbp) * RECT], P[:, :])
                        if pending:
                            eng, dst, src = pending.pop()
                            getattr(nc, eng).dma_start(dst, src)
                        ch_no += 1

                for eng, dst, src in pending:
                    getattr(nc, eng).dma_start(dst, src)
                pending = emit_extracts(
                    Brect[:, :], out[g * rg * W:(g + 1) * rg * W, :])

          for eng, dst, src in pending:
              getattr(nc, eng).dma_start(dst, src)

    nc.compile()
    return nc


def build_v5(rows=ROWS_PER_CORE, rg=64, lg=16, lt_bufs=2, rect_bufs=2,
             ps_bufs=4, load_eng="sync", ex_engs=("scalar", "gpsimd"),
             ev_engs=("vector", "scalar"), repeat=1, nbp=8, skip=()):
    """v4 + host-padded right ([C, rows*SEG], zeros in the 64-col pads) so
    loads are plain 2D DMAs with no on-chip memsets, and nbp blocks per psum
    tile on a 128-col grid (in-bank quadrants) with strided-src evicts."""
    import concourse.bass as bass
    import concourse.mybir as mybir
    import concourse.tile as tile
    from concourse import bacc

    nc = bacc.Bacc()
    left = nc.declare_dram_parameter("left", [C, rows * W], mybir.dt.bfloat16,
                                     isOutput=False)
    right = nc.declare_dram_parameter("right", [C, rows * SEG],
                                      mybir.dt.bfloat16, isOutput=False)
    out = nc.declare_dram_parameter("out", [rows * W, D], mybir.dt.bfloat16,
                                    isOutput=True)

    ng = rows // rg               # big rect groups
    nsub = rg // lg               # load subgroups per rect group
    nblocks = rg * NB             # rect blocks per group
    bcols = nblocks * RECT
    PB = 128                      # psum col grid per block

    with tile.TileContext(nc) as tc:
        with (
            tc.tile_pool(name="lt", bufs=lt_bufs) as lt_pool,
            tc.tile_pool(name="rt", bufs=lt_bufs) as rt_pool,
            tc.tile_pool(name="rect", bufs=rect_bufs) as rect_pool,
            tc.tile_pool(name="ps", bufs=ps_bufs, space="PSUM") as psum_pool,
        ):
          for _rep in range(repeat):
            for g in range(ng):
                Brect = rect_pool.tile([128, bcols], mybir.dt.bfloat16,
                                       tag="rect")
                for sub in range(nsub):
                    r0 = g * rg + sub * lg
                    Lt = lt_pool.tile([C, lg * W], mybir.dt.bfloat16, tag="lt")
                    Rt = rt_pool.tile([C, lg * SEG], mybir.dt.bfloat16,
                                      tag="rt")
                    if "loads" not in skip:
                        getattr(nc, load_eng).dma_start(
                            Lt[:, :], left[:, r0 * W:(r0 + lg) * W])
                        getattr(nc, load_eng).dma_start(
                            Rt[:, :], right[:, r0 * SEG:(r0 + lg) * SEG])

                    nchunks = (lg * NB) // nbp
                    for ch in range(nchunks):
                        P = psum_pool.tile([128, nbp * PB],
                                           mybir.dt.float32, tag="ps")
                        for bi in range(nbp):
                            rr, i = divmod(ch * nbp + bi, NB)
                            for s in range(2):
                                nc.tensor.matmul(
                                    P[64 * s:64 * (s + 1),
                                      bi * PB:bi * PB + RECT],
                                    Lt[:, rr * W + 128 * i + 64 * s:
                                       rr * W + 128 * i + 64 * (s + 1)],
                                    Rt[:, rr * SEG + 128 * i + 64 * s:
                                       rr * SEG + 128 * i + 64 * s + RECT],
                                    start=True, stop=True,
                                    tile_position=(0, 64 * s))
                        m0 = sub * lg * NB + ch * nbp
                        bap0 = Brect[:, m0 * RECT:(m0 + nbp) * RECT]
                        pap = P[:, :]
                        src = bass.AP(pap.tensor, pap.offset,
                                      [[nbp * PB, 128], [PB, nbp], [1, RECT]])
                        dst = bass.AP(bap0.tensor, bap0.offset,
                                      [[bcols, 128], [RECT, nbp], [1, RECT]])
                        ev = ev_engs[ch % len(ev_engs)]
                        if ev == "vector":
                            nc.vector.tensor_copy(dst, src)
                        else:
                            nc.scalar.copy(dst, src)

                # per-q band write: out[128m + q + 64h, d'] =
                # Brect[q + 64h, 127m + q + d']
                bap = Brect[:, :]
                oap = out[g * rg * W:(g + 1) * rg * W, :]
                for q in range(64 if "extract" not in skip else 0):
                    src = bass.AP(bap.tensor, bap.offset + q * bcols + q,
                                  [[64 * bcols, 2], [RECT, nblocks], [1, D]])
                    dst = bass.AP(oap.tensor, oap.offset + q * D,
                                  [[64 * D, 2], [128 * D, nblocks], [1, D]])
                    getattr(nc, ex_engs[q % len(ex_engs)]).dma_start(dst, src)

    nc.compile()
    return nc


def _to_bf16_t(x, scale, pad=0):
    """[rows, W, C] f32 -> [C, rows*(pad+W)] bf16, zeros in the pad cols."""
    import ml_dtypes
    if scale != 1.0:
        x = x * scale
    xt = np.ascontiguousarray(x.transpose(2, 0, 1)).astype(ml_dtypes.bfloat16)
    if pad:
        rows = xt.shape[1]
        padded = np.zeros((C, rows, pad + W), dtype=ml_dtypes.bfloat16)
        padded[:, :, pad:] = xt
        xt = padded
    return xt.reshape(C, -1)


def make_in_maps(seed=0, pad_right=False):
    """Random prepped per-core in_maps (bench harness helper)."""
    rng = np.random.default_rng(seed)
    lf = rng.standard_normal((ROWS, W, C), dtype=np.float32)
    rf = rng.standard_normal((ROWS, W, C), dtype=np.float32)
    in_maps = []
    for k in range(N_CORES):
        sl = slice(k * ROWS_PER_CORE, (k + 1) * ROWS_PER_CORE)
        in_maps.append({
            "left": _to_bf16_t(lf[sl], 1.0 / C),
            "right": _to_bf16_t(rf[sl], 1.0, pad=64 if pad_right else 0),
        })
    return in_maps


def in_map_to_rows(m):
    """Recover [ROWS_PER_CORE, W, C] f32 (prepped) arrays from an in_map."""
    lf = np.asarray(m["left"]).astype(np.float32)
    rf = np.asarray(m["right"]).astype(np.float32)
    lf = lf.reshape(C, ROWS_PER_CORE, W).transpose(1, 2, 0)
    seg = rf.size // (C * ROWS_PER_CORE)
    rf = rf.reshape(C, ROWS_PER_CORE, seg)[:, :, seg - W:].transpose(1, 2, 0)
    return lf, rf


_NC_CACHE = {}


def kernel(left_feature, right_feature):
    from concourse.bass_utils import run_bass_kernel_spmd

    lf = np.asarray(left_feature, dtype=np.float32).reshape(ROWS, W, C)
    rf = np.asarray(right_feature, dtype=np.float32).reshape(ROWS, W, C)

    if "nc" not in _NC_CACHE:
        _NC_CACHE["nc"] = build_v6(load_eng=("sync", "scalar"))
    nc = _NC_CACHE["nc"]

    in_maps = []
    for k in range(N_CORES):
        sl = slice(k * ROWS_PER_CORE, (k + 1) * ROWS_PER_CORE)
        in_maps.append({
            "left": _to_bf16_t(lf[sl], 1.0 / C),
            "right": _to_bf16_t(rf[sl], 1.0),
        })

    res = run_bass_kernel_spmd(nc, in_maps, core_ids=list(range(N_CORES)))

    out = np.empty((ROWS, W, D), dtype=np.float32)
    for k in range(N_CORES):
        g = res.results[k]["out"].astype(np.float32).reshape(
            ROWS_PER_CORE, W, D)
        out[k * ROWS_PER_CORE:(k + 1) * ROWS_PER_CORE] = g[:, :, ::-1]
    return out.reshape(B_FULL, H_FULL, W, D)
